# revision 27
# baseline (speedup 1.0000x reference)
"""Bidirectional DSS/Mamba block on 8 trn2 cores (Bass/Tile), v2.

Sharding: core = (batch b = core//2, d_inner half = core%2). Each core
computes the full in-proj for its batch (x is needed in full for x_proj),
scans its 256 d_inner channels in both directions, and produces a partial
(256-channel) contribution to the output projection; the host sums the two
partials per batch.

v2 layout: everything after the in-projection is PARITY-SPLIT along the
sequence: a [128, 900] working tile holds [even(450) | odd(450)] columns.
The sequential scan is radix-2 decimated: per (state n, 128-ch tile) the
forward recurrence over pairs  h[2k+1] = (a_e*a_o)[k] h[2k-1] + (a_o b_e +
b_o)[k]  runs as a 450-long tensor_tensor_scan (and the mirrored backward
one), halving DVE scan time.  Readout uses the C-trick: odd outputs are
C_o*Hf directly; even outputs are (C_e*a_e)*shift(Hf) plus a shared
w2*(sum_n C_n B_n) correction, so no h reconstruction is needed.

Elementwise work is n-pair blocked: one DVE op covers two states' worth of
[128, 1800] columns in packed bf16 (2x DVE rate). GPSIMD is useless on this
part (measured ~4-5us per op) so DVE does all elementwise work; ACT does
all exponentials (incl. softplus for dt).

The channel-tile (dtc) loop is OUTER so dtc=0's pooled-gate AllGather
(2 of them, split per dtc) is hidden under dtc=1's scan work. After the
last collective, g is folded into W_out by per-partition ACT scaling of
the weight tiles instead of rescaling the [128,900] activations.
"""

import os
import sys

sys.path.insert(0, "/opt/trn_rl_repo")

from contextlib import ExitStack

import ml_dtypes
import numpy as np

import concourse.bass as bass
import concourse.bacc as bacc
import concourse.tile as tile
from concourse import mybir
from concourse.bass_utils import run_bass_kernel_spmd

F32 = mybir.dt.float32
BF16 = mybir.dt.bfloat16
AF = mybir.ActivationFunctionType
OP = mybir.AluOpType

B, L, DM, DS, DI, R = 4, 900, 256, 16, 512, 16
DH = DI // 2          # d_inner channels per core
NDT = DH // 128       # 128-channel tiles per core (2)
H = L // 2            # 450, parity half length
FCH = [(0, 512), (512, L - 512)]  # PSUM-bank-aligned L chunks (in-proj)


def _bc_row(t, off, n):
    """Partition-broadcast AP: read n consecutive DRAM floats 128 times."""
    return bass.AP(tensor=t, offset=off, ap=[[0, 128], [1, n]])


def _build_v2(shared_a: bool):
    nc = bacc.Bacc("TRN2", num_devices=8)

    ein = lambda n, s: nc.dram_tensor(n, s, F32, kind="ExternalInput")
    ein_bf = lambda n, s: nc.dram_tensor(n, s, BF16, kind="ExternalInput")
    hsT = ein_bf("hsT", [DM, L])
    WinxT = ein_bf("WinxT", [DM, DI])
    WinzT = ein_bf("WinzT", [DM, DH])
    WxT = ein_bf("WxT", [DI, R + 2 * DS])
    WdtT = ein_bf("WdtT", [R, DH])
    bdt = ein("bdt", [128, NDT])
    Afc = ein("Afc", [128, NDT * DS])      # col dtc*16+n = A_f[own dtile, n]
    Abc = ein("Abc", [128, NDT * DS])
    Ddf = ein_bf("Ddf", [DH, 128])
    Ddb = ein_bf("Ddb", [DH, 128])
    I128 = ein_bf("I128", [128, 128])
    G2T = ein_bf("G2T", [2 * DI, 2 * DH])
    bgate2 = ein("bgate2", [1, 2 * DH])
    WoT = ein_bf("WoT", [2 * DH, DM])
    outp = nc.dram_tensor("outp", [DM, L], F32, kind="ExternalOutput")

    # bounce: rows 0..15 = B_n, 16..31 = C_n; each row = [even(450)|odd(450)]
    bc2 = nc.dram_tensor("bc2", [2 * DS, L], BF16, kind="Internal")
    cdb_d = nc.dram_tensor("cdb_d", [1, L], BF16, kind="Internal")
    ccw_i = nc.dram_tensor("ccw_i", [1, 8], F32, kind="Internal")
    ccw_o = nc.dram_tensor("ccw_o", [1, 16], F32, kind="Internal")
    cc_in = [nc.dram_tensor(f"cc_in{d}", [1, 2 * 128], BF16, kind="Internal")
             for d in range(NDT)]
    cc_out = [nc.dram_tensor(f"cc_out{d}", [1, 4 * 128], BF16, kind="Internal")
              for d in range(NDT)]
    g_dram = nc.dram_tensor("g_dram", [1, 4 * 128], F32, kind="Internal")

    with ExitStack() as ctx:
        tc = ctx.enter_context(tile.TileContext(nc))
        wpool = ctx.enter_context(tc.tile_pool(name="weights", bufs=1))
        apool = ctx.enter_context(tc.tile_pool(name="acts", bufs=1))

        def load(name, dram, p, f, eng=None):
            ts = []
            for i in range(0, p, 128):
                pp = min(128, p - i)
                t = wpool.tile([pp, f], dram.dtype, tag=f"{name}{i}", name=f"{name}{i}")
                (eng or nc.sync).dma_start(out=t, in_=dram[i : i + pp, :])
                ts.append(t)
            return ts

        # in-proj inputs first on the sync queue, interleaved so the first
        # matmul (winx0 x hs0-chunk0) can fire as early as possible
        winx = [wpool.tile([128, DI], BF16, tag=f"winx{i}", name=f"winx{i}")
                for i in range(2)]
        hs = [wpool.tile([128, L], BF16, tag=f"hs{i}", name=f"hs{i}")
              for i in range(2)]
        for kc in range(2):
            nc.sync.dma_start(out=winx[kc], in_=WinxT[kc * 128 : (kc + 1) * 128, :])
            nc.sync.dma_start(
                out=hs[kc][:, 0:512], in_=hsT[kc * 128 : (kc + 1) * 128, 0:512]
            )
        for kc in range(2):
            nc.sync.dma_start(
                out=hs[kc][:, 512:L], in_=hsT[kc * 128 : (kc + 1) * 128, 512:L]
            )
        winz = load("winz", WinzT, DM, DH)
        wx = load("wx", WxT, DI, R + 2 * DS, eng=nc.scalar)
        wdt = load("wdt", WdtT, R, DH, eng=nc.scalar)
        bdt_s = load("bdt", bdt, 128, NDT, eng=nc.scalar)[0]
        af_s = load("afc", Afc, 128, NDT * DS, eng=nc.scalar)[0]
        if not shared_a:
            ab_s = load("abc", Abc, 128, NDT * DS, eng=nc.scalar)[0]
        ddf = load("ddf", Ddf, DH, 128, eng=nc.gpsimd)
        ddb = load("ddb", Ddb, DH, 128, eng=nc.gpsimd)
        ident = load("ident", I128, 128, 128, eng=nc.gpsimd)[0]
        wo = load("wo", WoT, 2 * DH, DM, eng=nc.gpsimd)
        g2 = load("g2", G2T, 2 * DI, 2 * DH, eng=nc.gpsimd)
        bgate_r = load("bgate2", bgate2, 1, 2 * DH, eng=nc.gpsimd)[0]

        # warm up the CC stream early with a dummy 8-float AllGather
        ccw_t = apool.tile([1, 8], F32, tag="ccw", name="ccw")
        nc.vector.memset(ccw_t, 0.0)
        nc.sync.dma_start(out=ccw_i[:, :], in_=ccw_t)
        nc.gpsimd.collective_compute(
            "AllGather", OP.bypass,
            replica_groups=[[0, 1], [2, 3], [4, 5], [6, 7]],
            ins=[ccw_i[:, :]], outs=[ccw_o[:, :]],
        )

        # ---- in-proj -> parity-split silu(x) / silu(z) ----
        # xT[i], zg[i]: [128, 900] = [even(450) | odd(450)]
        xT = [apool.tile([128, L], BF16, tag=f"xT{i}", name=f"xT{i}") for i in range(4)]
        zg = [apool.tile([128, L], BF16, tag=f"zg{i}", name=f"zg{i}") for i in range(NDT)]
        dtT = [apool.tile([128, L], BF16, tag=f"dtT{i}", name=f"dtT{i}") for i in range(NDT)]
        dt2 = [apool.tile([128, H], BF16, tag=f"dt2{i}", name=f"dt2{i}") for i in range(NDT)]
        w2 = [apool.tile([128, L], BF16, tag=f"w2{i}", name=f"w2{i}") for i in range(NDT)]
        xdbl = apool.tile([R + 2 * DS, L], BF16, tag="xdbl", name="xdbl")
        cdbrep = apool.tile([128, L], BF16, tag="cdbrep", name="cdbrep")
        cw = [apool.tile([128, L], BF16, tag=f"cw{i}", name=f"cw{i}") for i in range(NDT)]

        with tc.tile_pool(name="ps_xz", bufs=2, space="PSUM") as ps_xz, \
             tc.tile_pool(name="ps_early", bufs=1, space="PSUM") as ps_early:
            def inproj(pc):
                ps = ps_xz.tile([128, L], F32, tag="xz", name="xz")
                for f0, fl in FCH:
                    for kc in range(2):
                        lhsT = (
                            winx[kc][:, pc * 128 : (pc + 1) * 128]
                            if pc < 4
                            else winz[kc][:, (pc - 4) * 128 : (pc - 3) * 128]
                        )
                        nc.tensor.matmul(
                            ps[:, f0 : f0 + fl], lhsT, hs[kc][:, f0 : f0 + fl],
                            start=(kc == 0), stop=(kc == 1),
                        )
                dst = xT[pc] if pc < 4 else zg[pc - 4]
                for par in range(2):
                    nc.scalar.activation(
                        dst[:, par * H : (par + 1) * H],
                        ps[:, par : L : 2], AF.Silu,
                    )

            for pc in range(4):          # x tiles first: x_proj needs them
                inproj(pc)

            # ---- x_proj -> xdbl [48, 900] parity-blocked; bounce B/C ----
            for par in range(2):
                psx = ps_early.tile([R + 2 * DS, H], F32, tag="aux", name="aux")
                for kc in range(4):
                    nc.tensor.matmul(
                        psx, wx[kc], xT[kc][:, par * H : (par + 1) * H],
                        start=(kc == 0), stop=(kc == 3),
                    )
                nc.scalar.activation(
                    xdbl[:, par * H : (par + 1) * H], psx, AF.Copy,
                )
            nc.sync.dma_start(out=bc2[:, :], in_=xdbl[R : R + 2 * DS, :])

            # ---- dt = softplus(dt_r @ WdtT + bdt) (parity halves) ----
            for dtc in range(NDT):
                for par in range(2):
                    psd = ps_early.tile([128, H], F32, tag="aux2", name="aux2")
                    nc.tensor.matmul(
                        psd,
                        wdt[0][:, dtc * 128 : (dtc + 1) * 128],
                        xdbl[0:R, par * H : (par + 1) * H],
                        start=True, stop=True,
                    )
                    # softplus(v+b) = ln(1 + exp(v+b)) in fp32
                    sp = apool.tile([128, H], F32, tag="sp_tmp", name="sp_tmp")
                    nc.scalar.activation(
                        sp, psd, AF.Exp, bias=bdt_s[:, dtc : dtc + 1]
                    )
                    nc.vector.tensor_scalar_add(sp, sp, 1.0)
                    nc.scalar.activation(
                        dtT[dtc][:, par * H : (par + 1) * H], sp, AF.Ln
                    )
                nc.vector.tensor_add(
                    dt2[dtc], dtT[dtc][:, 0:H], dtT[dtc][:, H:L]
                )
                nc.vector.tensor_mul(w2[dtc], dtT[dtc], xT[dtc])

            for pc in range(4, 6):       # z tiles: needed only for yg
                inproj(pc)

            # ---- CdotB = sum_n B_n*C_n (row), broadcast, w2-weighted ----
            tB = apool.tile([DS, L], BF16, tag="tB", name="tB")
            tC = apool.tile([DS, L], BF16, tag="tC", name="tC")
            nc.sync.dma_start(out=tB, in_=bc2[0:DS, :])
            nc.sync.dma_start(out=tC, in_=bc2[DS : 2 * DS, :])
            cb = apool.tile([DS, L], BF16, tag="cb", name="cb")
            nc.vector.tensor_mul(cb, tB, tC)
            ones16 = apool.tile([DS, 1], BF16, tag="ones16", name="ones16")
            nc.vector.memset(ones16, 1.0)
            ones11 = apool.tile([1, 1], F32, tag="ones11", name="ones11")
            nc.vector.memset(ones11, 1.0)
            cdb_row = apool.tile([1, L], BF16, tag="cdbrow", name="cdbrow")
            for par in range(2):
                ps_cb = ps_early.tile([1, H], F32, tag="pscb", name="pscb")
                nc.tensor.matmul(
                    ps_cb, ones16, cb[:, par * H : (par + 1) * H],
                    start=True, stop=True,
                )
                nc.scalar.activation(
                    cdb_row[:, par * H : (par + 1) * H], ps_cb, AF.Copy
                )
            nc.sync.dma_start(out=cdb_d[:, :], in_=cdb_row)
            nc.sync.dma_start(out=cdbrep, in_=_bc_row(cdb_d, 0, L))
            for dtc in range(NDT):
                nc.vector.tensor_mul(cw[dtc], w2[dtc], cdbrep)

        # ---- scan phase: dtc outer, n-pair inner ----
        yg = {}
        mcols = apool.tile([128, 4 * NDT], F32, tag="mcols", name="mcols")
        m2 = [apool.tile([128, 2], BF16, tag=f"m2_{i}", name=f"m2_{i}")
              for i in range(NDT)]
        u2c = [apool.tile([128, 4], BF16, tag=f"u2c{i}", name=f"u2c{i}")
               for i in range(NDT)]

        with tc.tile_pool(name="ps_g", bufs=1, space="PSUM") as ps_g:
            vps = ps_g.tile([1, 2 * DH], F32, tag="vps", name="vps")

            for dtc in range(NDT):
                for dr in range(2):
                    yg[(dr, dtc)] = apool.tile(
                        [128, L], BF16, tag=f"yg{dr}{dtc}", name=f"yg{dr}{dtc}"
                    )
                with tc.tile_pool(name=f"ps_y{dtc}", bufs=1, space="PSUM") as ps_y, \
                     tc.tile_pool(name=f"brep{dtc}", bufs=3) as brep_pool, \
                     tc.tile_pool(name=f"crep{dtc}", bufs=3) as crep_pool, \
                     tc.tile_pool(name=f"a4{dtc}", bufs=3) as a4_pool, \
                     tc.tile_pool(name=f"a2p{dtc}", bufs=3) as a2p_pool, \
                     tc.tile_pool(name=f"bb{dtc}", bufs=3) as bb_pool, \
                     tc.tile_pool(name=f"h4{dtc}", bufs=2) as h4_pool, \
                     tc.tile_pool(name=f"u{dtc}", bufs=2) as u_pool:
                    # 4 PSUM halves: fe, fo, be, bo
                    yp = {}
                    for dr in range(2):
                        for par in range(2):
                            yp[(dr, par)] = ps_y.tile(
                                [128, H], F32, tag=f"yp{dr}{par}", name=f"yp{dr}{par}"
                            )
                    # D-skip (start=True) + CdotB corrections
                    for dr in range(2):
                        dd = (ddf if dr == 0 else ddb)[dtc]
                        for par in range(2):
                            nc.tensor.matmul(
                                yp[(dr, par)], dd,
                                xT[dtc][:, par * H : (par + 1) * H],
                                start=True, stop=False, skip_group_check=True,
                            )
                    # fwd-even correction and bwd-odd correction
                    nc.tensor.matmul(
                        yp[(0, 0)], ident, cw[dtc][:, 0:H],
                        start=False, stop=False, skip_group_check=True,
                    )
                    nc.tensor.matmul(
                        yp[(1, 1)], ident, cw[dtc][:, H:L],
                        start=False, stop=False, skip_group_check=True,
                    )

                    def rework(apx, newap, doff=0):
                        return bass.AP(
                            tensor=apx.tensor, offset=apx.offset + doff,
                            ap=[apx.ap[0]] + newap,
                        )

                    for p in range(DS // 2):
                        n0 = 2 * p
                        last = p == DS // 2 - 1
                        brep = brep_pool.tile([128, 2 * L], BF16, tag="br", name="br")
                        crep = crep_pool.tile([128, 2 * L], BF16, tag="cr", name="cr")
                        nc.sync.dma_start(out=brep, in_=_bc_row(bc2, n0 * L, 2 * L))
                        nc.scalar.dma_start(
                            out=crep, in_=_bc_row(bc2, (DS + n0) * L, 2 * L)
                        )
                        # a4c: per n-block [a_b_e | a_f_o]  (scan-input a's)
                        # a4d: per n-block [a_f_e | a_b_o]  (readout a's)
                        # a2p: [A2f(n0)|A2f(n1)]; a2pb: backward pair products
                        a4c = a4_pool.tile([128, 2 * L], BF16, tag="a4c", name="a4c")
                        a2p = a2p_pool.tile([128, L], BF16, tag="a2p", name="a2p")
                        if shared_a:
                            a4d = a4c
                            a2pb = a2p
                            for k in range(2):
                                col = dtc * DS + n0 + k
                                nc.scalar.activation(
                                    a4c[:, k * L : (k + 1) * L], dtT[dtc], AF.Exp,
                                    scale=af_s[:, col : col + 1],
                                )
                                nc.scalar.activation(
                                    a2p[:, k * H : (k + 1) * H], dt2[dtc], AF.Exp,
                                    scale=af_s[:, col : col + 1],
                                )
                        else:
                            a4d = a4_pool.tile([128, 2 * L], BF16, tag="a4d", name="a4d")
                            a2pb = a2p_pool.tile([128, L], BF16, tag="a2pb", name="a2pb")
                            for k in range(2):
                                col = dtc * DS + n0 + k
                                kl = k * L
                                nc.scalar.activation(
                                    a4c[:, kl : kl + H], dtT[dtc][:, 0:H],
                                    AF.Exp, scale=ab_s[:, col : col + 1],
                                )
                                nc.scalar.activation(
                                    a4c[:, kl + H : kl + L], dtT[dtc][:, H:L],
                                    AF.Exp, scale=af_s[:, col : col + 1],
                                )
                                nc.scalar.activation(
                                    a4d[:, kl : kl + H], dtT[dtc][:, 0:H],
                                    AF.Exp, scale=af_s[:, col : col + 1],
                                )
                                nc.scalar.activation(
                                    a4d[:, kl + H : kl + L], dtT[dtc][:, H:L],
                                    AF.Exp, scale=ab_s[:, col : col + 1],
                                )
                                nc.scalar.activation(
                                    a2p[:, k * H : (k + 1) * H], dt2[dtc], AF.Exp,
                                    scale=af_s[:, col : col + 1],
                                )
                                nc.scalar.activation(
                                    a2pb[:, k * H : (k + 1) * H], dt2[dtc], AF.Exp,
                                    scale=ab_s[:, col : col + 1],
                                )
                        # b4 = w2 (repeated x2) * brep
                        b4 = bb_pool.tile([128, 2 * L], BF16, tag="b4", name="b4")
                        nc.vector.tensor_mul(
                            b4, rework(w2[dtc][:, 0:L], [[0, 2], [1, L]]), brep
                        )
                        # cross-parity view: per n-block swap [e|o] -> [o|e]
                        def cross(apx):
                            return rework(apx, [[L, 2], [-H, 2], [1, H]], doff=H)
                        tt4 = bb_pool.tile([128, 2 * L], BF16, tag="tt4", name="tt4")
                        nc.vector.tensor_mul(tt4, cross(a4c[:, 0 : 2 * L]), b4)
                        b24 = bb_pool.tile([128, 2 * L], BF16, tag="b24", name="b24")
                        nc.vector.tensor_add(b24, tt4, cross(b4[:, 0 : 2 * L]))
                        # scans: per n, fwd over [B2f], bwd (reversed) over [B2b]
                        h4 = h4_pool.tile([128, 2 * L], BF16, tag="h4", name="h4")
                        for k in range(2):
                            kl = k * L
                            nc.vector.tensor_tensor_scan(
                                h4[:, kl : kl + H],
                                a2p[:, k * H : (k + 1) * H],
                                b24[:, kl : kl + H], 0.0, OP.mult, OP.add,
                            )
                            nc.vector.tensor_tensor_scan(
                                h4[:, kl + L - 1 : (kl + H - 1) if kl + H - 1 >= 0 else None : -1],
                                a2pb[:, (k + 1) * H - 1 : (k * H - 1) if k * H - 1 >= 0 else None : -1],
                                b24[:, kl + L - 1 : kl + H - 1 : -1],
                                0.0, OP.mult, OP.add,
                            )
                        # readout: u13 = cross(Crep)*H4 ; aC4 = a4d*Crep ;
                        # u24 = shift(aC4)*shift(H4)
                        u13 = u_pool.tile([128, 2 * L], BF16, tag="u13", name="u13")
                        nc.vector.tensor_mul(u13, cross(crep[:, 0 : 2 * L]), h4)
                        ac4 = u_pool.tile([128, 2 * L], BF16, tag="ac4", name="ac4")
                        nc.vector.tensor_mul(ac4, a4d, crep)
                        u24 = u_pool.tile([128, 4 * (H - 1)], BF16, tag="u24", name="u24")
                        ac_sh = rework(
                            ac4[:, 0 : 2 * L], [[L, 2], [H - 1, 2], [1, H - 1]], doff=1
                        )
                        h4_sh = rework(
                            h4[:, 0 : 2 * L], [[L, 2], [H + 1, 2], [1, H - 1]]
                        )
                        nc.vector.tensor_mul(u24, ac_sh, h4_sh)
                        # PE accumulate
                        for k in range(2):
                            nc.tensor.matmul(
                                yp[(0, 1)], ident, u13[:, k * L : k * L + H],
                                start=False, stop=(last and k == 1),
                                skip_group_check=True,
                            )
                            nc.tensor.matmul(
                                yp[(1, 0)], ident, u13[:, k * L + H : (k + 1) * L],
                                start=False, stop=(last and k == 1),
                                skip_group_check=True,
                            )
                            o2 = 2 * k * (H - 1)
                            nc.tensor.matmul(
                                yp[(0, 0)][:, 1:H], ident,
                                u24[:, o2 : o2 + H - 1],
                                start=False, stop=(last and k == 1),
                                skip_group_check=True,
                            )
                            nc.tensor.matmul(
                                yp[(1, 1)][:, 0 : H - 1], ident,
                                u24[:, o2 + H - 1 : o2 + 2 * (H - 1)],
                                start=False, stop=(last and k == 1),
                                skip_group_check=True,
                            )

                    # ---- gate inputs: yg = y*zg with pooled accumulation ----
                    for dr in range(2):
                        for par in range(2):
                            nc.vector.scalar_tensor_tensor(
                                yg[(dr, dtc)][:, par * H : (par + 1) * H],
                                yp[(dr, par)], 1.0,
                                zg[dtc][:, par * H : (par + 1) * H],
                                OP.mult, OP.mult,
                                accum_out=mcols[:, 4 * dtc + 2 * dr + par :
                                                4 * dtc + 2 * dr + par + 1],
                            )
                # m2 = even+odd pooled parts; transpose on PE so the DRAM
                # DMA is 2 contiguous 128-element rows (not 256 scattered)
                nc.vector.tensor_add(
                    m2[dtc],
                    mcols[:, 4 * dtc : 4 * dtc + 4 : 2],
                    mcols[:, 4 * dtc + 1 : 4 * dtc + 4 : 2],
                )
                mt_ps = ps_g.tile([2, 128], BF16, tag="mt", name="mt")
                nc.tensor.matmul(mt_ps, m2[dtc], ident, is_transpose=True,
                                 skip_group_check=True)
                mt_sb = apool.tile([2, 128], BF16, tag=f"mts{dtc}", name=f"mts{dtc}")
                nc.vector.tensor_copy(mt_sb, mt_ps)
                nc.sync.dma_start(
                    out=bass.AP(tensor=cc_in[dtc], offset=0,
                                ap=[[128, 2], [1, 128]]),
                    in_=mt_sb,
                )
                nc.gpsimd.collective_compute(
                    "AllGather", OP.bypass,
                    replica_groups=[[0, 1], [2, 3], [4, 5], [6, 7]],
                    ins=[cc_in[dtc][:, :]], outs=[cc_out[dtc][:, :]],
                )
                ut_sb = apool.tile([4, 128], BF16, tag=f"uts{dtc}", name=f"uts{dtc}")
                nc.sync.dma_start(
                    out=ut_sb,
                    in_=bass.AP(tensor=cc_out[dtc], offset=0,
                                ap=[[128, 4], [1, 128]]),
                )
                ut_ps = ps_g.tile([128, 4], BF16, tag="ut", name="ut")
                nc.tensor.matmul(ut_ps, ut_sb, ident[0:4, 0:4], is_transpose=True,
                                 skip_group_check=True)
                nc.vector.tensor_copy(u2c[dtc], ut_ps)
                for j in range(4):
                    nc.tensor.matmul(
                        vps, u2c[dtc][:, j : j + 1], g2[dtc * 4 + j],
                        start=(dtc == 0 and j == 0), stop=(dtc == 1 and j == 3),
                        skip_group_check=True,
                    )

            # ---- gate: g = sigmoid(vps + bgate2); fold into W_out ----
            with tc.tile_pool(name="ps_tail", bufs=2, space="PSUM") as ps_tail:
                g_row = apool.tile([1, 2 * DH], F32, tag="grow", name="grow")
                nc.vector.tensor_add(g_row, vps, bgate_r)
                g_rowb = apool.tile([1, 2 * DH], F32, tag="growb", name="growb")
                nc.scalar.activation(g_rowb, g_row, AF.Sigmoid)
                # transpose g [1,512] -> [128,4] on the PE (no DRAM roundtrip)
                gps = ps_g.tile([128, 4], F32, tag="gps", name="gps")
                for kc in range(4):
                    nc.tensor.matmul(
                        gps[:, kc : kc + 1],
                        g_rowb[:, kc * 128 : (kc + 1) * 128],
                        ones11, is_transpose=True,
                        skip_group_check=True,
                    )
                g_sb = apool.tile([128, 4], F32, tag="g", name="g")
                nc.vector.tensor_copy(g_sb, gps)
                # scale yg by g on DVE (4x tensor_scalar; DVE is idle here)
                for kc in range(4):
                    t = yg[(kc // 2, kc % 2)]
                    nc.vector.tensor_scalar_mul(t, t, g_sb[:, kc : kc + 1])
                # out-proj: out[pc] = sum_kc wo[kc]^T yg[kc], parity-blocked
                # columns ([even(450) | odd(450)]); the host re-interleaves.
                out_sb = [apool.tile([128, L], F32, tag=f"o{i}", name=f"o{i}")
                          for i in range(2)]
                for pc in range(2):
                    for par in range(2):
                        ops_ = ps_tail.tile([128, H], F32, tag="ops", name="ops")
                        for kc in range(4):
                            nc.tensor.matmul(
                                ops_,
                                wo[kc][:, pc * 128 : (pc + 1) * 128],
                                yg[(kc // 2, kc % 2)][:, par * H : (par + 1) * H],
                                start=(kc == 0), stop=(kc == 3),
                            )
                        if par == 0:
                            nc.vector.tensor_copy(
                                out_sb[pc][:, par * H : (par + 1) * H], ops_
                            )
                        else:
                            nc.scalar.activation(
                                out_sb[pc][:, par * H : (par + 1) * H], ops_,
                                AF.Copy,
                            )
                        nc.sync.dma_start(
                            out=outp[pc * 128 : (pc + 1) * 128,
                                     par * H : (par + 1) * H],
                            in_=out_sb[pc][:, par * H : (par + 1) * H],
                        )

    nc.finalize()
    return nc


_NC_CACHE = {}


def _get_module(shared_a: bool):
    if shared_a not in _NC_CACHE:
        _NC_CACHE[shared_a] = _build_v2(shared_a)
    return _NC_CACHE[shared_a]


def _diag_stack(d):
    out = np.zeros((DH, 128), dtype=np.float32)
    for t in range(NDT):
        out[t * 128 : (t + 1) * 128, :] = np.diag(d[t * 128 : (t + 1) * 128])
    return out


def kernel(**inputs):
    inp = {k: np.asarray(v, dtype=np.float32) for k, v in inputs.items()}
    hs = inp["hidden_states"]
    W_in, W_x, W_dt = inp["W_in"], inp["W_xproj"], inp["W_dt"]
    b_dt = inp["b_dt"]
    A_f = -np.exp(inp["A_log_f"])      # (512, 16)
    A_b = -np.exp(inp["A_log_b"])
    D_f, D_b = inp["D_f"], inp["D_b"]
    W_g, b_g = inp["W_global"], inp["b_global"]
    W_gate, b_gate = inp["W_gate"], inp["b_gate"]
    W_out = inp["W_out"]

    shared_a = bool(np.array_equal(A_f, A_b))
    I = np.eye(128, dtype=np.float32)
    bf = ml_dtypes.bfloat16
    in_maps = []
    for core in range(8):
        b, h = core // 2, core % 2
        o = h * DH                      # own-half offset in d_inner
        perm = np.r_[o : o + DH, (DH - o) % DI : (DH - o) % DI + DH]  # own first
        ownc = np.r_[o : o + DH, DI + o : DI + o + DH]  # own rows of 2*DI concat
        # contraction row order for G2T: kc = dtc*4 + h'*2 + dr'
        ccorder2 = np.concatenate(
            [drp * 2 * DI // 2 + hp * DH + dtc * 128 + np.arange(128)
             for dtc in range(NDT) for hp in range(2) for drp in range(2)]
        )

        def acol(A):
            # [128, NDT*DS]: col (dtc*DS + n) = A[own dtile dtc, n]
            a = A[o : o + DH].reshape(NDT, 128, DS)
            return np.ascontiguousarray(a.transpose(1, 0, 2).reshape(128, NDT * DS))

        m = {
            "hsT": np.ascontiguousarray(hs[b].T).astype(bf),
            "WinxT": np.ascontiguousarray(W_in[:DI][perm].T).astype(bf),
            "WinzT": np.ascontiguousarray(W_in[DI + o : DI + o + DH].T).astype(bf),
            "WxT": np.ascontiguousarray(W_x[:, perm].T).astype(bf),
            "WdtT": np.ascontiguousarray(W_dt[o : o + DH].T).astype(bf),
            "bdt": np.ascontiguousarray(b_dt[o : o + DH].reshape(NDT, 128).T),
            "Afc": acol(A_f),
            "Abc": acol(A_b),
            "Ddf": _diag_stack(D_f[o : o + DH]).astype(bf),
            "Ddb": _diag_stack(D_b[o : o + DH]).astype(bf),
            "I128": I.astype(bf),
            "G2T": np.ascontiguousarray(
                (W_gate[ownc] @ W_g[:, ccorder2] / np.float32(L)).T
            ).astype(bf),
            "bgate2": np.ascontiguousarray(
                (b_gate[ownc] + W_gate[ownc] @ b_g).reshape(1, 512)
            ),
            "WoT": np.ascontiguousarray(W_out[:, ownc].T).astype(bf),
        }
        in_maps.append(m)

    nc = _get_module(shared_a)
    res = run_bass_kernel_spmd(nc, in_maps, core_ids=list(range(8)))
    outs = res.results
    out = np.zeros((B, L, DM), dtype=np.float32)
    for b in range(B):
        part = outs[2 * b]["outp"] + outs[2 * b + 1]["outp"]
        un = np.empty_like(part)          # columns are [even(450) | odd(450)]
        un[:, 0::2] = part[:, : L // 2]
        un[:, 1::2] = part[:, L // 2 :]
        out[b] = un.T
    return out


# revision 28
# speedup vs baseline: 1.0138x; 1.0138x over previous
"""Bidirectional DSS/Mamba block on 8 trn2 cores (Bass/Tile), v2.

Sharding: core = (batch b = core//2, d_inner half = core%2). Each core
computes the full in-proj for its batch (x is needed in full for x_proj),
scans its 256 d_inner channels in both directions, and produces a partial
(256-channel) contribution to the output projection; the host sums the two
partials per batch.

v2 layout: everything after the in-projection is PARITY-SPLIT along the
sequence: a [128, 900] working tile holds [even(450) | odd(450)] columns.
The sequential scan is radix-2 decimated: per (state n, 128-ch tile) the
forward recurrence over pairs  h[2k+1] = (a_e*a_o)[k] h[2k-1] + (a_o b_e +
b_o)[k]  runs as a 450-long tensor_tensor_scan (and the mirrored backward
one), halving DVE scan time.  Readout uses the C-trick: odd outputs are
C_o*Hf directly; even outputs are (C_e*a_e)*shift(Hf) plus a shared
w2*(sum_n C_n B_n) correction, so no h reconstruction is needed.

Elementwise work is n-pair blocked: one DVE op covers two states' worth of
[128, 1800] columns in packed bf16 (2x DVE rate). GPSIMD is useless on this
part (measured ~4-5us per op) so DVE does all elementwise work; ACT does
all exponentials (incl. softplus for dt).

The channel-tile (dtc) loop is OUTER so dtc=0's pooled-gate AllGather
(2 of them, split per dtc) is hidden under dtc=1's scan work. After the
last collective, g is folded into W_out by per-partition ACT scaling of
the weight tiles instead of rescaling the [128,900] activations.
"""

import os
import sys

sys.path.insert(0, "/opt/trn_rl_repo")

from contextlib import ExitStack

import ml_dtypes
import numpy as np

import concourse.bass as bass
import concourse.bacc as bacc
import concourse.tile as tile
from concourse import mybir
from concourse.bass_utils import run_bass_kernel_spmd

F32 = mybir.dt.float32
BF16 = mybir.dt.bfloat16
AF = mybir.ActivationFunctionType
OP = mybir.AluOpType

B, L, DM, DS, DI, R = 4, 900, 256, 16, 512, 16
DH = DI // 2          # d_inner channels per core
NDT = DH // 128       # 128-channel tiles per core (2)
H = L // 2            # 450, parity half length
FCH = [(0, 512), (512, L - 512)]  # PSUM-bank-aligned L chunks (in-proj)


def _bc_row(t, off, n):
    """Partition-broadcast AP: read n consecutive DRAM floats 128 times."""
    return bass.AP(tensor=t, offset=off, ap=[[0, 128], [1, n]])


def _build_v2(shared_a: bool):
    nc = bacc.Bacc("TRN2", num_devices=8)

    ein = lambda n, s: nc.dram_tensor(n, s, F32, kind="ExternalInput")
    ein_bf = lambda n, s: nc.dram_tensor(n, s, BF16, kind="ExternalInput")
    hsT = ein_bf("hsT", [DM, L])
    WinxT = ein_bf("WinxT", [DM, DI])
    WinzT = ein_bf("WinzT", [DM, DH])
    WxT = ein_bf("WxT", [DI, R + 2 * DS])
    WdtT = ein_bf("WdtT", [R, DH])
    bdt = ein("bdt", [128, NDT])
    Afc = ein("Afc", [128, NDT * DS])      # col dtc*16+n = A_f[own dtile, n]
    Abc = ein("Abc", [128, NDT * DS])
    Ddf = ein_bf("Ddf", [DH, 128])
    Ddb = ein_bf("Ddb", [DH, 128])
    I128 = ein_bf("I128", [128, 128])
    G2T = ein_bf("G2T", [2 * DI, 2 * DH])
    bgate2 = ein("bgate2", [1, 2 * DH])
    WoT = ein_bf("WoT", [2 * DH, DM])
    outp = nc.dram_tensor("outp", [DM, L], F32, kind="ExternalOutput")

    # bounce: rows 0..15 = B_n, 16..31 = C_n; each row = [even(450)|odd(450)]
    bc2 = nc.dram_tensor("bc2", [2 * DS, L], BF16, kind="Internal")
    cdb_d = nc.dram_tensor("cdb_d", [1, L], BF16, kind="Internal")
    ccw_i = nc.dram_tensor("ccw_i", [1, 8], F32, kind="Internal")
    ccw_o = nc.dram_tensor("ccw_o", [1, 16], F32, kind="Internal")
    cc_in = [nc.dram_tensor(f"cc_in{d}", [1, 2 * 128], BF16, kind="Internal")
             for d in range(NDT)]
    cc_out = [nc.dram_tensor(f"cc_out{d}", [1, 4 * 128], BF16, kind="Internal")
              for d in range(NDT)]
    g_dram = nc.dram_tensor("g_dram", [1, 4 * 128], F32, kind="Internal")

    with ExitStack() as ctx:
        tc = ctx.enter_context(tile.TileContext(nc))
        wpool = ctx.enter_context(tc.tile_pool(name="weights", bufs=1))
        apool = ctx.enter_context(tc.tile_pool(name="acts", bufs=1))

        def load(name, dram, p, f, eng=None):
            ts = []
            for i in range(0, p, 128):
                pp = min(128, p - i)
                t = wpool.tile([pp, f], dram.dtype, tag=f"{name}{i}", name=f"{name}{i}")
                (eng or nc.sync).dma_start(out=t, in_=dram[i : i + pp, :])
                ts.append(t)
            return ts

        # in-proj inputs first on the sync queue, interleaved so the first
        # matmul (winx0 x hs0-chunk0) can fire as early as possible
        winx = [wpool.tile([128, DI], BF16, tag=f"winx{i}", name=f"winx{i}")
                for i in range(2)]
        hs = [wpool.tile([128, L], BF16, tag=f"hs{i}", name=f"hs{i}")
              for i in range(2)]
        for kc in range(2):
            nc.sync.dma_start(out=winx[kc], in_=WinxT[kc * 128 : (kc + 1) * 128, :])
            nc.scalar.dma_start(
                out=hs[kc][:, 0:512], in_=hsT[kc * 128 : (kc + 1) * 128, 0:512]
            )
        for kc in range(2):
            nc.scalar.dma_start(
                out=hs[kc][:, 512:L], in_=hsT[kc * 128 : (kc + 1) * 128, 512:L]
            )
        winz = load("winz", WinzT, DM, DH)
        wx = load("wx", WxT, DI, R + 2 * DS, eng=nc.scalar)
        wdt = load("wdt", WdtT, R, DH, eng=nc.scalar)
        bdt_s = load("bdt", bdt, 128, NDT, eng=nc.scalar)[0]
        af_s = load("afc", Afc, 128, NDT * DS, eng=nc.scalar)[0]
        if not shared_a:
            ab_s = load("abc", Abc, 128, NDT * DS, eng=nc.scalar)[0]
        ddf = load("ddf", Ddf, DH, 128, eng=nc.gpsimd)
        ddb = load("ddb", Ddb, DH, 128, eng=nc.gpsimd)
        ident = load("ident", I128, 128, 128, eng=nc.gpsimd)[0]
        wo = load("wo", WoT, 2 * DH, DM, eng=nc.gpsimd)
        g2 = load("g2", G2T, 2 * DI, 2 * DH, eng=nc.gpsimd)
        bgate_r = load("bgate2", bgate2, 1, 2 * DH, eng=nc.gpsimd)[0]

        # warm up the CC stream early with a dummy 8-float AllGather
        ccw_t = apool.tile([1, 8], F32, tag="ccw", name="ccw")
        nc.vector.memset(ccw_t, 0.0)
        nc.sync.dma_start(out=ccw_i[:, :], in_=ccw_t)
        nc.gpsimd.collective_compute(
            "AllGather", OP.bypass,
            replica_groups=[[0, 1], [2, 3], [4, 5], [6, 7]],
            ins=[ccw_i[:, :]], outs=[ccw_o[:, :]],
        )

        # ---- in-proj -> parity-split silu(x) / silu(z) ----
        # xT[i], zg[i]: [128, 900] = [even(450) | odd(450)]
        xT = [apool.tile([128, L], BF16, tag=f"xT{i}", name=f"xT{i}") for i in range(4)]
        zg = [apool.tile([128, L], BF16, tag=f"zg{i}", name=f"zg{i}") for i in range(NDT)]
        dtT = [apool.tile([128, L], BF16, tag=f"dtT{i}", name=f"dtT{i}") for i in range(NDT)]
        dt2 = [apool.tile([128, H], BF16, tag=f"dt2{i}", name=f"dt2{i}") for i in range(NDT)]
        w2 = [apool.tile([128, L], BF16, tag=f"w2{i}", name=f"w2{i}") for i in range(NDT)]
        xdbl = apool.tile([R + 2 * DS, L], BF16, tag="xdbl", name="xdbl")
        cdbrep = apool.tile([128, L], BF16, tag="cdbrep", name="cdbrep")
        cw = [apool.tile([128, L], BF16, tag=f"cw{i}", name=f"cw{i}") for i in range(NDT)]

        with tc.tile_pool(name="ps_xz", bufs=2, space="PSUM") as ps_xz, \
             tc.tile_pool(name="ps_early", bufs=1, space="PSUM") as ps_early:
            def inproj(pc):
                ps = ps_xz.tile([128, L], F32, tag="xz", name="xz")
                for f0, fl in FCH:
                    for kc in range(2):
                        lhsT = (
                            winx[kc][:, pc * 128 : (pc + 1) * 128]
                            if pc < 4
                            else winz[kc][:, (pc - 4) * 128 : (pc - 3) * 128]
                        )
                        nc.tensor.matmul(
                            ps[:, f0 : f0 + fl], lhsT, hs[kc][:, f0 : f0 + fl],
                            start=(kc == 0), stop=(kc == 1),
                        )
                dst = xT[pc] if pc < 4 else zg[pc - 4]
                for par in range(2):
                    nc.scalar.activation(
                        dst[:, par * H : (par + 1) * H],
                        ps[:, par : L : 2], AF.Silu,
                    )

            for pc in range(4):          # x tiles first: x_proj needs them
                inproj(pc)

            # ---- x_proj -> xdbl [48, 900] parity-blocked; bounce B/C ----
            for par in range(2):
                psx = ps_early.tile([R + 2 * DS, H], F32, tag="aux", name="aux")
                for kc in range(4):
                    nc.tensor.matmul(
                        psx, wx[kc], xT[kc][:, par * H : (par + 1) * H],
                        start=(kc == 0), stop=(kc == 3),
                    )
                nc.scalar.activation(
                    xdbl[:, par * H : (par + 1) * H], psx, AF.Copy,
                )
            nc.sync.dma_start(out=bc2[:, :], in_=xdbl[R : R + 2 * DS, :])

            # ---- dt = softplus(dt_r @ WdtT + bdt) (parity halves) ----
            for dtc in range(NDT):
                for par in range(2):
                    psd = ps_early.tile([128, H], F32, tag="aux2", name="aux2")
                    nc.tensor.matmul(
                        psd,
                        wdt[0][:, dtc * 128 : (dtc + 1) * 128],
                        xdbl[0:R, par * H : (par + 1) * H],
                        start=True, stop=True,
                    )
                    # softplus(v+b) = ln(1 + exp(v+b)) in fp32
                    sp = apool.tile([128, H], F32, tag="sp_tmp", name="sp_tmp")
                    nc.scalar.activation(
                        sp, psd, AF.Exp, bias=bdt_s[:, dtc : dtc + 1]
                    )
                    nc.vector.tensor_scalar_add(sp, sp, 1.0)
                    nc.scalar.activation(
                        dtT[dtc][:, par * H : (par + 1) * H], sp, AF.Ln
                    )
                nc.vector.tensor_add(
                    dt2[dtc], dtT[dtc][:, 0:H], dtT[dtc][:, H:L]
                )
                nc.vector.tensor_mul(w2[dtc], dtT[dtc], xT[dtc])

            for pc in range(4, 6):       # z tiles: needed only for yg
                inproj(pc)

            # ---- CdotB = sum_n B_n*C_n (row), broadcast, w2-weighted ----
            tB = apool.tile([DS, L], BF16, tag="tB", name="tB")
            tC = apool.tile([DS, L], BF16, tag="tC", name="tC")
            nc.sync.dma_start(out=tB, in_=bc2[0:DS, :])
            nc.sync.dma_start(out=tC, in_=bc2[DS : 2 * DS, :])
            cb = apool.tile([DS, L], BF16, tag="cb", name="cb")
            nc.vector.tensor_mul(cb, tB, tC)
            ones16 = apool.tile([DS, 1], BF16, tag="ones16", name="ones16")
            nc.vector.memset(ones16, 1.0)
            ones11 = apool.tile([1, 1], F32, tag="ones11", name="ones11")
            nc.vector.memset(ones11, 1.0)
            cdb_row = apool.tile([1, L], BF16, tag="cdbrow", name="cdbrow")
            for par in range(2):
                ps_cb = ps_early.tile([1, H], F32, tag="pscb", name="pscb")
                nc.tensor.matmul(
                    ps_cb, ones16, cb[:, par * H : (par + 1) * H],
                    start=True, stop=True,
                )
                nc.scalar.activation(
                    cdb_row[:, par * H : (par + 1) * H], ps_cb, AF.Copy
                )
            nc.sync.dma_start(out=cdb_d[:, :], in_=cdb_row)
            nc.sync.dma_start(out=cdbrep, in_=_bc_row(cdb_d, 0, L))
            for dtc in range(NDT):
                nc.vector.tensor_mul(cw[dtc], w2[dtc], cdbrep)

        # ---- scan phase: dtc outer, n-pair inner ----
        yg = {}
        mcols = apool.tile([128, 4 * NDT], F32, tag="mcols", name="mcols")
        m2 = [apool.tile([128, 2], BF16, tag=f"m2_{i}", name=f"m2_{i}")
              for i in range(NDT)]
        u2c = [apool.tile([128, 4], BF16, tag=f"u2c{i}", name=f"u2c{i}")
               for i in range(NDT)]

        with tc.tile_pool(name="ps_g", bufs=1, space="PSUM") as ps_g:
            vps = ps_g.tile([1, 2 * DH], F32, tag="vps", name="vps")

            for dtc in range(NDT):
                for dr in range(2):
                    yg[(dr, dtc)] = apool.tile(
                        [128, L], BF16, tag=f"yg{dr}{dtc}", name=f"yg{dr}{dtc}"
                    )
                with tc.tile_pool(name=f"ps_y{dtc}", bufs=1, space="PSUM") as ps_y, \
                     tc.tile_pool(name=f"brep{dtc}", bufs=3) as brep_pool, \
                     tc.tile_pool(name=f"crep{dtc}", bufs=3) as crep_pool, \
                     tc.tile_pool(name=f"a4{dtc}", bufs=3) as a4_pool, \
                     tc.tile_pool(name=f"a2p{dtc}", bufs=3) as a2p_pool, \
                     tc.tile_pool(name=f"bb{dtc}", bufs=3) as bb_pool, \
                     tc.tile_pool(name=f"h4{dtc}", bufs=2) as h4_pool, \
                     tc.tile_pool(name=f"u{dtc}", bufs=2) as u_pool:
                    # 4 PSUM halves: fe, fo, be, bo
                    yp = {}
                    for dr in range(2):
                        for par in range(2):
                            yp[(dr, par)] = ps_y.tile(
                                [128, H], F32, tag=f"yp{dr}{par}", name=f"yp{dr}{par}"
                            )
                    # D-skip (start=True) + CdotB corrections
                    for dr in range(2):
                        dd = (ddf if dr == 0 else ddb)[dtc]
                        for par in range(2):
                            nc.tensor.matmul(
                                yp[(dr, par)], dd,
                                xT[dtc][:, par * H : (par + 1) * H],
                                start=True, stop=False, skip_group_check=True,
                            )
                    # fwd-even correction and bwd-odd correction
                    nc.tensor.matmul(
                        yp[(0, 0)], ident, cw[dtc][:, 0:H],
                        start=False, stop=False, skip_group_check=True,
                    )
                    nc.tensor.matmul(
                        yp[(1, 1)], ident, cw[dtc][:, H:L],
                        start=False, stop=False, skip_group_check=True,
                    )

                    def rework(apx, newap, doff=0):
                        return bass.AP(
                            tensor=apx.tensor, offset=apx.offset + doff,
                            ap=[apx.ap[0]] + newap,
                        )

                    for p in range(DS // 2):
                        n0 = 2 * p
                        last = p == DS // 2 - 1
                        brep = brep_pool.tile([128, 2 * L], BF16, tag="br", name="br")
                        crep = crep_pool.tile([128, 2 * L], BF16, tag="cr", name="cr")
                        nc.sync.dma_start(out=brep, in_=_bc_row(bc2, n0 * L, 2 * L))
                        nc.scalar.dma_start(
                            out=crep, in_=_bc_row(bc2, (DS + n0) * L, 2 * L)
                        )
                        # a4c: per n-block [a_b_e | a_f_o]  (scan-input a's)
                        # a4d: per n-block [a_f_e | a_b_o]  (readout a's)
                        # a2p: [A2f(n0)|A2f(n1)]; a2pb: backward pair products
                        a4c = a4_pool.tile([128, 2 * L], BF16, tag="a4c", name="a4c")
                        a2p = a2p_pool.tile([128, L], BF16, tag="a2p", name="a2p")
                        if shared_a:
                            a4d = a4c
                            a2pb = a2p
                            for k in range(2):
                                col = dtc * DS + n0 + k
                                nc.scalar.activation(
                                    a4c[:, k * L : (k + 1) * L], dtT[dtc], AF.Exp,
                                    scale=af_s[:, col : col + 1],
                                )
                                nc.scalar.activation(
                                    a2p[:, k * H : (k + 1) * H], dt2[dtc], AF.Exp,
                                    scale=af_s[:, col : col + 1],
                                )
                        else:
                            a4d = a4_pool.tile([128, 2 * L], BF16, tag="a4d", name="a4d")
                            a2pb = a2p_pool.tile([128, L], BF16, tag="a2pb", name="a2pb")
                            for k in range(2):
                                col = dtc * DS + n0 + k
                                kl = k * L
                                nc.scalar.activation(
                                    a4c[:, kl : kl + H], dtT[dtc][:, 0:H],
                                    AF.Exp, scale=ab_s[:, col : col + 1],
                                )
                                nc.scalar.activation(
                                    a4c[:, kl + H : kl + L], dtT[dtc][:, H:L],
                                    AF.Exp, scale=af_s[:, col : col + 1],
                                )
                                nc.scalar.activation(
                                    a4d[:, kl : kl + H], dtT[dtc][:, 0:H],
                                    AF.Exp, scale=af_s[:, col : col + 1],
                                )
                                nc.scalar.activation(
                                    a4d[:, kl + H : kl + L], dtT[dtc][:, H:L],
                                    AF.Exp, scale=ab_s[:, col : col + 1],
                                )
                                nc.scalar.activation(
                                    a2p[:, k * H : (k + 1) * H], dt2[dtc], AF.Exp,
                                    scale=af_s[:, col : col + 1],
                                )
                                nc.scalar.activation(
                                    a2pb[:, k * H : (k + 1) * H], dt2[dtc], AF.Exp,
                                    scale=ab_s[:, col : col + 1],
                                )
                        # b4 = w2 (repeated x2) * brep
                        b4 = bb_pool.tile([128, 2 * L], BF16, tag="b4", name="b4")
                        nc.vector.tensor_mul(
                            b4, rework(w2[dtc][:, 0:L], [[0, 2], [1, L]]), brep
                        )
                        # cross-parity view: per n-block swap [e|o] -> [o|e]
                        def cross(apx):
                            return rework(apx, [[L, 2], [-H, 2], [1, H]], doff=H)
                        tt4 = bb_pool.tile([128, 2 * L], BF16, tag="tt4", name="tt4")
                        nc.vector.tensor_mul(tt4, cross(a4c[:, 0 : 2 * L]), b4)
                        b24 = bb_pool.tile([128, 2 * L], BF16, tag="b24", name="b24")
                        nc.vector.tensor_add(b24, tt4, cross(b4[:, 0 : 2 * L]))
                        # scans: per n, fwd over [B2f], bwd (reversed) over [B2b]
                        h4 = h4_pool.tile([128, 2 * L], BF16, tag="h4", name="h4")
                        for k in range(2):
                            kl = k * L
                            nc.vector.tensor_tensor_scan(
                                h4[:, kl : kl + H],
                                a2p[:, k * H : (k + 1) * H],
                                b24[:, kl : kl + H], 0.0, OP.mult, OP.add,
                            )
                            nc.vector.tensor_tensor_scan(
                                h4[:, kl + L - 1 : (kl + H - 1) if kl + H - 1 >= 0 else None : -1],
                                a2pb[:, (k + 1) * H - 1 : (k * H - 1) if k * H - 1 >= 0 else None : -1],
                                b24[:, kl + L - 1 : kl + H - 1 : -1],
                                0.0, OP.mult, OP.add,
                            )
                        # readout: u13 = cross(Crep)*H4 ; aC4 = a4d*Crep ;
                        # u24 = shift(aC4)*shift(H4)
                        u13 = u_pool.tile([128, 2 * L], BF16, tag="u13", name="u13")
                        nc.vector.tensor_mul(u13, cross(crep[:, 0 : 2 * L]), h4)
                        ac4 = u_pool.tile([128, 2 * L], BF16, tag="ac4", name="ac4")
                        nc.vector.tensor_mul(ac4, a4d, crep)
                        u24 = u_pool.tile([128, 4 * (H - 1)], BF16, tag="u24", name="u24")
                        ac_sh = rework(
                            ac4[:, 0 : 2 * L], [[L, 2], [H - 1, 2], [1, H - 1]], doff=1
                        )
                        h4_sh = rework(
                            h4[:, 0 : 2 * L], [[L, 2], [H + 1, 2], [1, H - 1]]
                        )
                        nc.vector.tensor_mul(u24, ac_sh, h4_sh)
                        # PE accumulate
                        for k in range(2):
                            nc.tensor.matmul(
                                yp[(0, 1)], ident, u13[:, k * L : k * L + H],
                                start=False, stop=(last and k == 1),
                                skip_group_check=True,
                            )
                            nc.tensor.matmul(
                                yp[(1, 0)], ident, u13[:, k * L + H : (k + 1) * L],
                                start=False, stop=(last and k == 1),
                                skip_group_check=True,
                            )
                            o2 = 2 * k * (H - 1)
                            nc.tensor.matmul(
                                yp[(0, 0)][:, 1:H], ident,
                                u24[:, o2 : o2 + H - 1],
                                start=False, stop=(last and k == 1),
                                skip_group_check=True,
                            )
                            nc.tensor.matmul(
                                yp[(1, 1)][:, 0 : H - 1], ident,
                                u24[:, o2 + H - 1 : o2 + 2 * (H - 1)],
                                start=False, stop=(last and k == 1),
                                skip_group_check=True,
                            )

                    # ---- gate inputs: yg = y*zg with pooled accumulation ----
                    for dr in range(2):
                        for par in range(2):
                            nc.vector.scalar_tensor_tensor(
                                yg[(dr, dtc)][:, par * H : (par + 1) * H],
                                yp[(dr, par)], 1.0,
                                zg[dtc][:, par * H : (par + 1) * H],
                                OP.mult, OP.mult,
                                accum_out=mcols[:, 4 * dtc + 2 * dr + par :
                                                4 * dtc + 2 * dr + par + 1],
                            )
                # m2 = even+odd pooled parts; transpose on PE so the DRAM
                # DMA is 2 contiguous 128-element rows (not 256 scattered)
                nc.vector.tensor_add(
                    m2[dtc],
                    mcols[:, 4 * dtc : 4 * dtc + 4 : 2],
                    mcols[:, 4 * dtc + 1 : 4 * dtc + 4 : 2],
                )
                mt_ps = ps_g.tile([2, 128], BF16, tag="mt", name="mt")
                nc.tensor.matmul(mt_ps, m2[dtc], ident, is_transpose=True,
                                 skip_group_check=True)
                mt_sb = apool.tile([2, 128], BF16, tag=f"mts{dtc}", name=f"mts{dtc}")
                nc.vector.tensor_copy(mt_sb, mt_ps)
                nc.sync.dma_start(
                    out=bass.AP(tensor=cc_in[dtc], offset=0,
                                ap=[[128, 2], [1, 128]]),
                    in_=mt_sb,
                )
                nc.gpsimd.collective_compute(
                    "AllGather", OP.bypass,
                    replica_groups=[[0, 1], [2, 3], [4, 5], [6, 7]],
                    ins=[cc_in[dtc][:, :]], outs=[cc_out[dtc][:, :]],
                )
                ut_sb = apool.tile([4, 128], BF16, tag=f"uts{dtc}", name=f"uts{dtc}")
                nc.sync.dma_start(
                    out=ut_sb,
                    in_=bass.AP(tensor=cc_out[dtc], offset=0,
                                ap=[[128, 4], [1, 128]]),
                )
                ut_ps = ps_g.tile([128, 4], BF16, tag="ut", name="ut")
                nc.tensor.matmul(ut_ps, ut_sb, ident[0:4, 0:4], is_transpose=True,
                                 skip_group_check=True)
                nc.vector.tensor_copy(u2c[dtc], ut_ps)
                for j in range(4):
                    nc.tensor.matmul(
                        vps, u2c[dtc][:, j : j + 1], g2[dtc * 4 + j],
                        start=(dtc == 0 and j == 0), stop=(dtc == 1 and j == 3),
                        skip_group_check=True,
                    )

            # ---- gate: g = sigmoid(vps + bgate2); fold into W_out ----
            with tc.tile_pool(name="ps_tail", bufs=2, space="PSUM") as ps_tail:
                g_row = apool.tile([1, 2 * DH], F32, tag="grow", name="grow")
                nc.vector.tensor_add(g_row, vps, bgate_r)
                g_rowb = apool.tile([1, 2 * DH], F32, tag="growb", name="growb")
                nc.scalar.activation(g_rowb, g_row, AF.Sigmoid)
                # transpose g [1,512] -> [128,4] on the PE (no DRAM roundtrip)
                gps = ps_g.tile([128, 4], F32, tag="gps", name="gps")
                for kc in range(4):
                    nc.tensor.matmul(
                        gps[:, kc : kc + 1],
                        g_rowb[:, kc * 128 : (kc + 1) * 128],
                        ones11, is_transpose=True,
                        skip_group_check=True,
                    )
                g_sb = apool.tile([128, 4], F32, tag="g", name="g")
                nc.vector.tensor_copy(g_sb, gps)
                # scale yg by g on DVE (4x tensor_scalar; DVE is idle here)
                for kc in range(4):
                    t = yg[(kc // 2, kc % 2)]
                    nc.vector.tensor_scalar_mul(t, t, g_sb[:, kc : kc + 1])
                # out-proj: out[pc] = sum_kc wo[kc]^T yg[kc], parity-blocked
                # columns ([even(450) | odd(450)]); the host re-interleaves.
                out_sb = [apool.tile([128, L], F32, tag=f"o{i}", name=f"o{i}")
                          for i in range(2)]
                for pc in range(2):
                    for par in range(2):
                        ops_ = ps_tail.tile([128, H], F32, tag="ops", name="ops")
                        for kc in range(4):
                            nc.tensor.matmul(
                                ops_,
                                wo[kc][:, pc * 128 : (pc + 1) * 128],
                                yg[(kc // 2, kc % 2)][:, par * H : (par + 1) * H],
                                start=(kc == 0), stop=(kc == 3),
                            )
                        if par == 0:
                            nc.vector.tensor_copy(
                                out_sb[pc][:, par * H : (par + 1) * H], ops_
                            )
                        else:
                            nc.scalar.activation(
                                out_sb[pc][:, par * H : (par + 1) * H], ops_,
                                AF.Copy,
                            )
                        nc.sync.dma_start(
                            out=outp[pc * 128 : (pc + 1) * 128,
                                     par * H : (par + 1) * H],
                            in_=out_sb[pc][:, par * H : (par + 1) * H],
                        )

    nc.finalize()
    return nc


_NC_CACHE = {}


def _get_module(shared_a: bool):
    if shared_a not in _NC_CACHE:
        _NC_CACHE[shared_a] = _build_v2(shared_a)
    return _NC_CACHE[shared_a]


def _diag_stack(d):
    out = np.zeros((DH, 128), dtype=np.float32)
    for t in range(NDT):
        out[t * 128 : (t + 1) * 128, :] = np.diag(d[t * 128 : (t + 1) * 128])
    return out


def kernel(**inputs):
    inp = {k: np.asarray(v, dtype=np.float32) for k, v in inputs.items()}
    hs = inp["hidden_states"]
    W_in, W_x, W_dt = inp["W_in"], inp["W_xproj"], inp["W_dt"]
    b_dt = inp["b_dt"]
    A_f = -np.exp(inp["A_log_f"])      # (512, 16)
    A_b = -np.exp(inp["A_log_b"])
    D_f, D_b = inp["D_f"], inp["D_b"]
    W_g, b_g = inp["W_global"], inp["b_global"]
    W_gate, b_gate = inp["W_gate"], inp["b_gate"]
    W_out = inp["W_out"]

    shared_a = bool(np.array_equal(A_f, A_b))
    I = np.eye(128, dtype=np.float32)
    bf = ml_dtypes.bfloat16
    in_maps = []
    for core in range(8):
        b, h = core // 2, core % 2
        o = h * DH                      # own-half offset in d_inner
        perm = np.r_[o : o + DH, (DH - o) % DI : (DH - o) % DI + DH]  # own first
        ownc = np.r_[o : o + DH, DI + o : DI + o + DH]  # own rows of 2*DI concat
        # contraction row order for G2T: kc = dtc*4 + h'*2 + dr'
        ccorder2 = np.concatenate(
            [drp * 2 * DI // 2 + hp * DH + dtc * 128 + np.arange(128)
             for dtc in range(NDT) for hp in range(2) for drp in range(2)]
        )

        def acol(A):
            # [128, NDT*DS]: col (dtc*DS + n) = A[own dtile dtc, n]
            a = A[o : o + DH].reshape(NDT, 128, DS)
            return np.ascontiguousarray(a.transpose(1, 0, 2).reshape(128, NDT * DS))

        m = {
            "hsT": np.ascontiguousarray(hs[b].T).astype(bf),
            "WinxT": np.ascontiguousarray(W_in[:DI][perm].T).astype(bf),
            "WinzT": np.ascontiguousarray(W_in[DI + o : DI + o + DH].T).astype(bf),
            "WxT": np.ascontiguousarray(W_x[:, perm].T).astype(bf),
            "WdtT": np.ascontiguousarray(W_dt[o : o + DH].T).astype(bf),
            "bdt": np.ascontiguousarray(b_dt[o : o + DH].reshape(NDT, 128).T),
            "Afc": acol(A_f),
            "Abc": acol(A_b),
            "Ddf": _diag_stack(D_f[o : o + DH]).astype(bf),
            "Ddb": _diag_stack(D_b[o : o + DH]).astype(bf),
            "I128": I.astype(bf),
            "G2T": np.ascontiguousarray(
                (W_gate[ownc] @ W_g[:, ccorder2] / np.float32(L)).T
            ).astype(bf),
            "bgate2": np.ascontiguousarray(
                (b_gate[ownc] + W_gate[ownc] @ b_g).reshape(1, 512)
            ),
            "WoT": np.ascontiguousarray(W_out[:, ownc].T).astype(bf),
        }
        in_maps.append(m)

    nc = _get_module(shared_a)
    res = run_bass_kernel_spmd(nc, in_maps, core_ids=list(range(8)))
    outs = res.results
    out = np.zeros((B, L, DM), dtype=np.float32)
    for b in range(B):
        part = outs[2 * b]["outp"] + outs[2 * b + 1]["outp"]
        un = np.empty_like(part)          # columns are [even(450) | odd(450)]
        un[:, 0::2] = part[:, : L // 2]
        un[:, 1::2] = part[:, L // 2 :]
        out[b] = un.T
    return out
